# revision 8
# baseline (speedup 1.0000x reference)
"""Multi-headed attention (B=4, S=2048, D=1024, H=16) on 8 trn2 NeuronCores.

Sharding: core c handles batch b=c//2 and head-group g=c%2 (8 heads = 512 of
the 1024 projection output dims; data parallel over B, tensor parallel over
heads).  Each core computes its heads' Q/K/V projections, masked-softmax
attention, and a partial output projection (contraction over its 512
head-dims).  The host sums the two partials per batch and adds bo.

Layout strategy (all matmuls bf16 with fp32 PSUM accumulation):
  - activations kept feature-major (transposed): QT/KT [j, s]; V seq-major
    [s, j] with a ones-column appended per head so the A@V matmul emits the
    softmax denominator as a 65th output row for free.
  - scoresT [ks, q] per head; 2 heads run concurrently on the PE via
    tile_position row-tiling (K=64 each).
  - Exp on ScalarE straight out of PSUM (scale=1/sqrt(64) folded in; scores
    are O(1) so no max-subtraction is needed); mask applied multiplicatively
    on VectorE in bf16.
  - attention output OT [65, q] is PE-transposed to [q, 65], normalized by
    the reciprocal denominator (per-partition scalar), and PE-transposed
    back so the output projection can contract over head-dims.
"""

import json
import numpy as np
import ml_dtypes

import concourse.bass as bass
import concourse.mybir as mybir
import concourse.tile as tile
from concourse.bass_utils import run_bass_kernel_spmd
from concourse.masks import make_identity

BF16 = mybir.dt.bfloat16
F32 = mybir.dt.float32
NP_BF16 = ml_dtypes.bfloat16

B, S, D, H = 4, 2048, 1024, 16
HD = 64          # head dim
G = 2            # head groups (cores per batch)
HPC = H // G     # 8 heads per core
JC = HPC * HD    # 512 local projection dims per core
NK = D // 128    # 8 contraction chunks
NJT = JC // 128  # 4 j-tiles (head pairs)
NSC = S // 512   # 4 s/q groups of 512
NST = S // 128   # 16 s/ks tiles of 128

_cache = {}


# --------------------------------------------------------------------------
# walrus in this container encodes at most ONE sync-wait command per
# instruction; the Tile tail drain carries several.  Split them at the BIR
# JSON level into single-wait drains.
def _split_multi_waits(bir_json_bytes):
    d = json.loads(bir_json_bytes)

    def fix_block(bb):
        insts = bb.get("instructions")
        if not isinstance(insts, list):
            return
        out = []
        for ins in insts:
            si = ins.get("sync_info") or {}
            waits = si.get("on_wait") or []
            if len(waits) > 1:
                for i, w in enumerate(waits[:-1]):
                    out.append({
                        "debug": ins.get("debug"),
                        "engine": ins["engine"],
                        "ins": [],
                        "name": f"{ins['name']}-w{i}",
                        "opcode": "Drain",
                        "outs": [],
                        "sync_info": {"on_update": [], "on_wait": [w]},
                    })
                si["on_wait"] = [waits[-1]]
            out.append(ins)
        bb["instructions"] = out

    def walk(o):
        if isinstance(o, dict):
            if "instructions" in o:
                fix_block(o)
            for v in o.values():
                walk(v)
        elif isinstance(o, list):
            for v in o:
                walk(v)

    walk(d)
    return json.dumps(d).encode()


def _install_birfix():
    import concourse.bass_utils as bu
    if getattr(bu, "_birfix_installed", False):
        return
    orig = bu.compile_bir_kernel

    def patched(bir_json, tmpdir, neff_name="file.neff"):
        if isinstance(bir_json, str):
            bir_json = bir_json.encode()
        return orig(_split_multi_waits(bir_json), tmpdir, neff_name)

    bu.compile_bir_kernel = patched
    try:
        import concourse.bass2jax as b2j
        b2j.compile_bir_kernel = patched
    except Exception:
        pass
    bu._birfix_installed = True


# --------------------------------------------------------------------------
def build_program():
    nc = bass.Bass()
    Exp = mybir.ActivationFunctionType.Exp
    Ident = mybir.ActivationFunctionType.Identity

    xq = nc.declare_dram_parameter("xq", [NSC, NK, 128, 512], BF16, isOutput=False)
    xk = nc.declare_dram_parameter("xk", [NSC, NK, 128, 512], BF16, isOutput=False)
    xv = nc.declare_dram_parameter("xv", [NSC, NK, 128, 512], BF16, isOutput=False)
    mf = nc.declare_dram_parameter("mf", [NSC, NST, 128, 1024], BF16, isOutput=False)
    wq = nc.declare_dram_parameter("wq", [NK, 128, JC], BF16, isOutput=False)
    wk = nc.declare_dram_parameter("wk", [NK, 128, JC], BF16, isOutput=False)
    wv = nc.declare_dram_parameter("wv", [NK, 128, JC], BF16, isOutput=False)
    wo = nc.declare_dram_parameter("wo", [NJT, 128, D], BF16, isOutput=False)
    bq = nc.declare_dram_parameter("bq", [128, NJT], F32, isOutput=False)
    bk = nc.declare_dram_parameter("bk", [128, NJT], F32, isOutput=False)
    bv = nc.declare_dram_parameter("bv", [1, JC], BF16, isOutput=False)
    yp = nc.declare_dram_parameter("yp", [S, D], F32, isOutput=True)

    with tile.TileContext(nc) as tc:
        with tc.tile_pool(name="singles", bufs=1) as singles:
            qt_sb = [singles.tile([128, S], BF16, tag=f"qt{j}", name=f"qt{j}") for j in range(NJT)]
            kt_sb = [singles.tile([128, S], BF16, tag=f"kt{j}", name=f"kt{j}") for j in range(NJT)]
            v_sb = [singles.tile([128, HPC, HD + 1], BF16, tag=f"v{t}", name=f"v{t}") for t in range(NST)]
            ot_sb = [singles.tile([128, S], BF16, tag=f"ot{j}", name=f"ot{j}") for j in range(NJT)]
            wq_sb = singles.tile([128, NK, JC], BF16, tag="wq")
            wk_sb = singles.tile([128, NK, JC], BF16, tag="wk")
            wv_sb = singles.tile([128, NK, JC], BF16, tag="wv")
            wo_sb = singles.tile([128, NJT, D], BF16, tag="wo")
            bq_sb = singles.tile([128, NJT], F32, tag="bq")
            bk_sb = singles.tile([128, NJT], F32, tag="bk")
            bv_sb = singles.tile([1, JC], BF16, tag="bv")
            ones_sb = singles.tile([1, 128], BF16, tag="ones")
            id32_sb = singles.tile([128, 128], F32, tag="id32")
            id16_sb = singles.tile([128, 128], BF16, tag="id16")

            nc.sync.dma_start(out=wv_sb, in_=wv.rearrange("k p n -> p k n"))
            nc.sync.dma_start(out=wk_sb, in_=wk.rearrange("k p n -> p k n"))
            nc.sync.dma_start(out=wq_sb, in_=wq.rearrange("k p n -> p k n"))
            nc.sync.dma_start(out=wo_sb, in_=wo.rearrange("c p n -> p c n"))
            nc.sync.dma_start(out=bq_sb, in_=bq[:, :])
            nc.sync.dma_start(out=bk_sb, in_=bk[:, :])
            nc.sync.dma_start(out=bv_sb, in_=bv[:, :])
            nc.vector.memset(ones_sb, 1.0)
            make_identity(nc, id32_sb)
            make_identity(nc, id16_sb)
            for t in range(NST):
                nc.vector.memset(v_sb[t][:, :, HD:HD + 1], 1.0)

            # ---------------- phase 1: projections ----------------
            # V first (attention needs all of it), then K/Q by j-tile so the
            # first head-pair's attention can start while later pairs project.
            with (
                tc.tile_pool(name="xv_p", bufs=4) as xv_pool,
                tc.tile_pool(name="xq_p", bufs=4 * NK) as xq_pool,
                tc.tile_pool(name="xk_p", bufs=4 * NK) as xk_pool,
                tc.tile_pool(name="pj_ps", bufs=8, space="PSUM") as pj_ps,
            ):
                for sc in range(NSC):
                    ps = [pj_ps.tile([128, 512], F32, tag="pj", name="pj") for _ in range(4)]
                    for k in range(NK):
                        xt = xv_pool.tile([128, 512], BF16, tag="xv", name="xvt")
                        nc.sync.dma_start(out=xt, in_=xv[sc, k])
                        for sl in range(4):
                            nc.tensor.matmul(
                                ps[sl], xt[:, sl * 128:(sl + 1) * 128], wv_sb[:, k, :],
                                start=(k == 0), stop=False)
                    for sl in range(4):
                        nc.tensor.matmul(ps[sl], ones_sb, bv_sb, start=False, stop=True)
                        nc.vector.tensor_copy(
                            out=v_sb[sc * 4 + sl][:, :, 0:HD], in_=ps[sl])

                # load all xq/xk tiles once; loop j-tiles outermost
                xqt = {}
                xkt = {}
                for sc in range(NSC):
                    for k in range(NK):
                        t1 = xk_pool.tile([128, 512], BF16, tag="xk", name="xkt")
                        nc.sync.dma_start(out=t1, in_=xk[sc, k])
                        xkt[sc, k] = t1
                        t2 = xq_pool.tile([128, 512], BF16, tag="xq", name="xqt")
                        nc.sync.dma_start(out=t2, in_=xq[sc, k])
                        xqt[sc, k] = t2
                for jt in range(NJT):
                    for w_t, b_t, dst, xtiles in (
                        (wk_sb, bk_sb, kt_sb, xkt),
                        (wq_sb, bq_sb, qt_sb, xqt),
                    ):
                        for sc in range(NSC):
                            ps = pj_ps.tile([128, 512], F32, tag="pj", name="pj")
                            for k in range(NK):
                                nc.tensor.matmul(
                                    ps, w_t[:, k, jt * 128:(jt + 1) * 128],
                                    xtiles[sc, k],
                                    start=(k == 0), stop=(k == NK - 1))
                            nc.vector.tensor_scalar_add(
                                dst[jt][:, sc * 512:(sc + 1) * 512], ps,
                                b_t[:, jt:jt + 1])

            # ---------------- phase 2: attention (+ output projection) -----
            with (
                tc.tile_pool(name="m", bufs=18) as m_pool,
                tc.tile_pool(name="p", bufs=8) as p_pool,
                tc.tile_pool(name="aux_sb", bufs=4) as aux_sb,
                tc.tile_pool(name="y_sb", bufs=2) as y_sb,
                tc.tile_pool(name="sc_ps", bufs=1, space="PSUM") as sc_ps,
                tc.tile_pool(name="av_ps", bufs=2, space="PSUM") as av_ps,
                tc.tile_pool(name="aux_ps", bufs=2, space="PSUM") as aux_ps,
            ):
                for qg in range(NSC):
                    msk = []
                    for kst in range(NST):
                        mt = m_pool.tile([128, 1024], BF16, tag="m", name="mt")
                        nc.sync.dma_start(out=mt, in_=mf[qg, kst])
                        msk.append(mt)
                    for pr in range(NJT):
                        qa = qt_sb[pr][0:64, qg * 512:(qg + 1) * 512]
                        qb = qt_sb[pr][64:128, qg * 512:(qg + 1) * 512]
                        pts = []
                        for kg in range(NST // 2):  # two ks-tiles per group
                            sab = sc_ps.tile([128, 2048], F32, tag="s", name="sab")
                            pt = p_pool.tile([128, 2048], BF16, tag="p", name="pt")
                            for u in range(2):
                                kst = 2 * kg + u
                                ka = kt_sb[pr][0:64, kst * 128:(kst + 1) * 128]
                                kb = kt_sb[pr][64:128, kst * 128:(kst + 1) * 128]
                                nc.tensor.matmul(sab[:, u * 1024:u * 1024 + 512],
                                                 ka, qa, tile_position=(0, 0))
                                nc.tensor.matmul(sab[:, u * 1024 + 512:(u + 1) * 1024],
                                                 kb, qb, tile_position=(64, 0))
                            nc.scalar.activation(out=pt, in_=sab, func=Exp, scale=0.125)
                            nc.vector.tensor_mul(pt[:, 0:1024], pt[:, 0:1024],
                                                 msk[2 * kg])
                            nc.vector.tensor_mul(pt[:, 1024:2048], pt[:, 1024:2048],
                                                 msk[2 * kg + 1])
                            pts.append(pt)
                        # A@V with denominator row (V ones-column)
                        otps = []
                        for hh in range(2):
                            h = 2 * pr + hh
                            ot = av_ps.tile([HD + 1, 512], F32, tag="av", name="ot")
                            for kst in range(NST):
                                nc.tensor.matmul(
                                    ot, v_sb[kst][:, h, :],
                                    pts[kst // 2][:, (kst % 2) * 1024 + hh * 512:
                                                  (kst % 2) * 1024 + (hh + 1) * 512],
                                    start=(kst == 0), stop=(kst == NST - 1))
                            otf = aux_sb.tile([HD + 1, 512], F32, tag="otf", name="otf")
                            nc.vector.tensor_copy(out=otf, in_=ot)
                            otps.append(otf)
                        # transpose -> normalize -> transpose back, 128 q at a time
                        for i in range(4):
                            onp = aux_sb.tile([128, 2 * HD], BF16, tag="on", name="onp")
                            for hh in range(2):
                                op = aux_ps.tile([128, HD + 1], F32, tag="aux", name="op")
                                nc.tensor.transpose(
                                    op, otps[hh][:, i * 128:(i + 1) * 128],
                                    id32_sb[0:HD + 1, 0:HD + 1])
                                rden = aux_sb.tile([128, 1], F32, tag="rd", name="rden")
                                nc.vector.reciprocal(out=rden, in_=op[:, HD:HD + 1])
                                nc.vector.tensor_scalar_mul(
                                    onp[:, hh * HD:(hh + 1) * HD], op[:, 0:HD], rden)
                            bk_ps = aux_ps.tile([128, 128], BF16, tag="aux", name="bkp")
                            nc.tensor.transpose(bk_ps, onp, id16_sb)
                            col = qg * 512 + i * 128
                            nc.vector.tensor_copy(
                                out=ot_sb[pr][:, col:col + 128], in_=bk_ps)
                    # output projection for this q-group (overlaps next group)
                    for qi in range(4):
                        qt = qg * 4 + qi
                        for nh in range(2):
                            ps = aux_ps.tile([128, 512], F32, tag="aux", name="yps")
                            for pr in range(NJT):
                                nc.tensor.matmul(
                                    ps, ot_sb[pr][:, qt * 128:(qt + 1) * 128],
                                    wo_sb[:, pr, nh * 512:(nh + 1) * 512],
                                    start=(pr == 0), stop=(pr == NJT - 1))
                            ysb = y_sb.tile([128, 512], F32, tag="ysb", name="ysb")
                            nc.vector.tensor_copy(out=ysb, in_=ps)
                            nc.sync.dma_start(
                                out=yp[qt * 128:(qt + 1) * 128,
                                       nh * 512:(nh + 1) * 512],
                                in_=ysb)

    return nc


# --------------------------------------------------------------------------
def _prep_inputs(inputs):
    """Slice/transpose/cast the full inputs into 8 per-core input maps."""
    q_in = np.asarray(inputs["query_input"], np.float32)
    k_in = np.asarray(inputs["key_input"], np.float32)
    v_in = np.asarray(inputs["value_input"], np.float32)
    mask = np.asarray(inputs["mask"])  # [B, 1, S, S] bool
    Wq = np.asarray(inputs["Wq"], np.float32)
    Wk = np.asarray(inputs["Wk"], np.float32)
    Wv = np.asarray(inputs["Wv"], np.float32)
    Wo = np.asarray(inputs["Wo"], np.float32)
    bqf = np.asarray(inputs["bq"], np.float32)
    bkf = np.asarray(inputs["bk"], np.float32)
    bvf = np.asarray(inputs["bv"], np.float32)

    def tile_xt(x_b):
        # [S, D] -> x.T tiled [NSC, NK, 128, 512]
        xt = np.ascontiguousarray(x_b.T).astype(NP_BF16)        # [D, S]
        return np.ascontiguousarray(
            xt.reshape(NK, 128, NSC, 512).transpose(2, 0, 1, 3))

    xts = {n: [tile_xt(a[b]) for b in range(B)]
           for n, a in (("xq", q_in), ("xk", k_in), ("xv", v_in))}
    mfs = []
    for b in range(B):
        m = (~mask[b, 0]).T.astype(NP_BF16)                     # [ks, q]
        mt = m.reshape(NST, 128, NSC, 512).transpose(2, 0, 1, 3)
        mfs.append(np.ascontiguousarray(
            np.concatenate([mt, mt], axis=3)))              # duplicated for 2 heads

    in_maps = []
    for c in range(8):
        b, g = c // 2, c % 2
        js = slice(g * JC, (g + 1) * JC)
        in_maps.append({
            "xq": xts["xq"][b],
            "xk": xts["xk"][b],
            "xv": xts["xv"][b],
            "mf": mfs[b],
            "wq": np.ascontiguousarray(Wq[js, :].T).astype(NP_BF16).reshape(NK, 128, JC),
            "wk": np.ascontiguousarray(Wk[js, :].T).astype(NP_BF16).reshape(NK, 128, JC),
            "wv": np.ascontiguousarray(Wv[js, :].T).astype(NP_BF16).reshape(NK, 128, JC),
            "wo": np.ascontiguousarray(Wo[:, js].T).astype(NP_BF16).reshape(NJT, 128, D),
            "bq": np.ascontiguousarray(bqf[js].reshape(NJT, 128).T),
            "bk": np.ascontiguousarray(bkf[js].reshape(NJT, 128).T),
            "bv": np.ascontiguousarray(bvf[js].astype(NP_BF16).reshape(1, JC)),
        })
    return in_maps


def kernel(**inputs):
    _install_birfix()
    if "nc" not in _cache:
        _cache["nc"] = build_program()
    nc = _cache["nc"]
    in_maps = _prep_inputs(inputs)
    res = run_bass_kernel_spmd(nc, in_maps, core_ids=list(range(8)))
    bo = np.asarray(inputs["bo"], np.float32)
    out = np.empty((B, S, D), np.float32)
    for b in range(B):
        out[b] = res.results[2 * b]["yp"] + res.results[2 * b + 1]["yp"] + bo[None, :]
    return out


# revision 9
# speedup vs baseline: 1.0174x; 1.0174x over previous
"""Multi-headed attention (B=4, S=2048, D=1024, H=16) on 8 trn2 NeuronCores.

Sharding: core c handles batch b=c//2 and head-group g=c%2 (8 heads = 512 of
the 1024 projection output dims; data parallel over B, tensor parallel over
heads).  Each core computes its heads' Q/K/V projections, masked-softmax
attention, and a partial output projection (contraction over its 512
head-dims).  The host sums the two partials per batch and adds bo.

Layout strategy (all matmuls bf16 with fp32 PSUM accumulation):
  - activations kept feature-major (transposed): QT/KT [j, s]; V seq-major
    [s, j] with a ones-column appended per head so the A@V matmul emits the
    softmax denominator as a 65th output row for free.
  - scoresT [ks, q] per head; 2 heads run concurrently on the PE via
    tile_position row-tiling (K=64 each).
  - Exp on ScalarE straight out of PSUM (scale=1/sqrt(64) folded in; scores
    are O(1) so no max-subtraction is needed); mask applied multiplicatively
    on VectorE in bf16.
  - attention output OT [65, q] is PE-transposed to [q, 65], normalized by
    the reciprocal denominator (per-partition scalar), and PE-transposed
    back so the output projection can contract over head-dims.
"""

import json
import numpy as np
import ml_dtypes

import concourse.bass as bass
import concourse.mybir as mybir
import concourse.tile as tile
from concourse.bass_utils import run_bass_kernel_spmd
from concourse.masks import make_identity

BF16 = mybir.dt.bfloat16
F32 = mybir.dt.float32
NP_BF16 = ml_dtypes.bfloat16

B, S, D, H = 4, 2048, 1024, 16
HD = 64          # head dim
G = 2            # head groups (cores per batch)
HPC = H // G     # 8 heads per core
JC = HPC * HD    # 512 local projection dims per core
NK = D // 128    # 8 contraction chunks
NJT = JC // 128  # 4 j-tiles (head pairs)
NSC = S // 512   # 4 s/q groups of 512
NST = S // 128   # 16 s/ks tiles of 128

_cache = {}


# --------------------------------------------------------------------------
# walrus in this container encodes at most ONE sync-wait command per
# instruction; the Tile tail drain carries several.  Split them at the BIR
# JSON level into single-wait drains.
def _split_multi_waits(bir_json_bytes):
    d = json.loads(bir_json_bytes)

    def fix_block(bb):
        insts = bb.get("instructions")
        if not isinstance(insts, list):
            return
        out = []
        for ins in insts:
            si = ins.get("sync_info") or {}
            waits = si.get("on_wait") or []
            if len(waits) > 1:
                for i, w in enumerate(waits[:-1]):
                    out.append({
                        "debug": ins.get("debug"),
                        "engine": ins["engine"],
                        "ins": [],
                        "name": f"{ins['name']}-w{i}",
                        "opcode": "Drain",
                        "outs": [],
                        "sync_info": {"on_update": [], "on_wait": [w]},
                    })
                si["on_wait"] = [waits[-1]]
            out.append(ins)
        bb["instructions"] = out

    def walk(o):
        if isinstance(o, dict):
            if "instructions" in o:
                fix_block(o)
            for v in o.values():
                walk(v)
        elif isinstance(o, list):
            for v in o:
                walk(v)

    walk(d)
    return json.dumps(d).encode()


def _install_birfix():
    import concourse.bass_utils as bu
    if getattr(bu, "_birfix_installed", False):
        return
    orig = bu.compile_bir_kernel

    def patched(bir_json, tmpdir, neff_name="file.neff"):
        if isinstance(bir_json, str):
            bir_json = bir_json.encode()
        return orig(_split_multi_waits(bir_json), tmpdir, neff_name)

    bu.compile_bir_kernel = patched
    try:
        import concourse.bass2jax as b2j
        b2j.compile_bir_kernel = patched
    except Exception:
        pass
    bu._birfix_installed = True


# --------------------------------------------------------------------------
def build_program():
    nc = bass.Bass()
    Exp = mybir.ActivationFunctionType.Exp
    Ident = mybir.ActivationFunctionType.Identity

    xq = nc.declare_dram_parameter("xq", [NSC, NK, 128, 512], BF16, isOutput=False)
    xk = nc.declare_dram_parameter("xk", [NSC, NK, 128, 512], BF16, isOutput=False)
    xv = nc.declare_dram_parameter("xv", [NSC, NK, 128, 512], BF16, isOutput=False)
    mf = nc.declare_dram_parameter("mf", [NSC, NST, 128, 1024], BF16, isOutput=False)
    wq = nc.declare_dram_parameter("wq", [NK, 128, JC], BF16, isOutput=False)
    wk = nc.declare_dram_parameter("wk", [NK, 128, JC], BF16, isOutput=False)
    wv = nc.declare_dram_parameter("wv", [NK, 128, JC], BF16, isOutput=False)
    wo = nc.declare_dram_parameter("wo", [NJT, 128, D], BF16, isOutput=False)
    bq = nc.declare_dram_parameter("bq", [128, NJT], F32, isOutput=False)
    bk = nc.declare_dram_parameter("bk", [128, NJT], F32, isOutput=False)
    bv = nc.declare_dram_parameter("bv", [1, JC], BF16, isOutput=False)
    yp = nc.declare_dram_parameter("yp", [S, D], F32, isOutput=True)

    with tile.TileContext(nc) as tc:
        with tc.tile_pool(name="singles", bufs=1) as singles:
            qt_sb = [singles.tile([128, S], BF16, tag=f"qt{j}", name=f"qt{j}") for j in range(NJT)]
            kt_sb = [singles.tile([128, S], BF16, tag=f"kt{j}", name=f"kt{j}") for j in range(NJT)]
            v_sb = [singles.tile([128, HPC, HD + 1], BF16, tag=f"v{t}", name=f"v{t}") for t in range(NST)]
            ot_sb = [singles.tile([128, S], BF16, tag=f"ot{j}", name=f"ot{j}") for j in range(NJT)]
            wq_sb = singles.tile([128, NK, JC], BF16, tag="wq")
            wk_sb = singles.tile([128, NK, JC], BF16, tag="wk")
            wv_sb = singles.tile([128, NK, JC], BF16, tag="wv")
            wo_sb = singles.tile([128, NJT, D], BF16, tag="wo")
            bq_sb = singles.tile([128, NJT], F32, tag="bq")
            bk_sb = singles.tile([128, NJT], F32, tag="bk")
            bv_sb = singles.tile([1, JC], BF16, tag="bv")
            ones_sb = singles.tile([1, 128], BF16, tag="ones")
            id32_sb = singles.tile([128, 128], F32, tag="id32")
            id16_sb = singles.tile([128, 128], BF16, tag="id16")

            nc.sync.dma_start(out=wv_sb, in_=wv.rearrange("k p n -> p k n"))
            nc.sync.dma_start(out=wk_sb, in_=wk.rearrange("k p n -> p k n"))
            nc.sync.dma_start(out=wq_sb, in_=wq.rearrange("k p n -> p k n"))
            nc.sync.dma_start(out=wo_sb, in_=wo.rearrange("c p n -> p c n"))
            nc.sync.dma_start(out=bq_sb, in_=bq[:, :])
            nc.sync.dma_start(out=bk_sb, in_=bk[:, :])
            nc.sync.dma_start(out=bv_sb, in_=bv[:, :])
            nc.vector.memset(ones_sb, 1.0)
            make_identity(nc, id32_sb)
            make_identity(nc, id16_sb)
            for t in range(NST):
                nc.vector.memset(v_sb[t][:, :, HD:HD + 1], 1.0)

            # ---------------- phase 1: projections ----------------
            # V first (attention needs all of it), then K/Q by j-tile so the
            # first head-pair's attention can start while later pairs project.
            with (
                tc.tile_pool(name="xv_p", bufs=4) as xv_pool,
                tc.tile_pool(name="xq_p", bufs=4 * NK) as xq_pool,
                tc.tile_pool(name="xk_p", bufs=4 * NK) as xk_pool,
                tc.tile_pool(name="pj_ps", bufs=8, space="PSUM") as pj_ps,
            ):
                for sc in range(NSC):
                    ps = [pj_ps.tile([128, 512], F32, tag="pj", name="pj") for _ in range(4)]
                    for k in range(NK):
                        xt = xv_pool.tile([128, 512], BF16, tag="xv", name="xvt")
                        nc.sync.dma_start(out=xt, in_=xv[sc, k])
                        for sl in range(4):
                            nc.tensor.matmul(
                                ps[sl], xt[:, sl * 128:(sl + 1) * 128], wv_sb[:, k, :],
                                start=(k == 0), stop=False)
                    for sl in range(4):
                        nc.tensor.matmul(ps[sl], ones_sb, bv_sb, start=False, stop=True)
                        nc.vector.tensor_copy(
                            out=v_sb[sc * 4 + sl][:, :, 0:HD], in_=ps[sl])

                # load all xq/xk tiles once; loop j-tiles outermost
                xqt = {}
                xkt = {}
                for sc in range(NSC):
                    for k in range(NK):
                        t1 = xk_pool.tile([128, 512], BF16, tag="xk", name="xkt")
                        nc.sync.dma_start(out=t1, in_=xk[sc, k])
                        xkt[sc, k] = t1
                        t2 = xq_pool.tile([128, 512], BF16, tag="xq", name="xqt")
                        nc.sync.dma_start(out=t2, in_=xq[sc, k])
                        xqt[sc, k] = t2
                for jt in range(NJT):
                    for w_t, b_t, dst, xtiles in (
                        (wk_sb, bk_sb, kt_sb, xkt),
                        (wq_sb, bq_sb, qt_sb, xqt),
                    ):
                        for sc in range(NSC):
                            ps = pj_ps.tile([128, 512], F32, tag="pj", name="pj")
                            for k in range(NK):
                                nc.tensor.matmul(
                                    ps, w_t[:, k, jt * 128:(jt + 1) * 128],
                                    xtiles[sc, k],
                                    start=(k == 0), stop=(k == NK - 1))
                            nc.vector.tensor_scalar_add(
                                dst[jt][:, sc * 512:(sc + 1) * 512], ps,
                                b_t[:, jt:jt + 1])

            # ---------------- phase 2: attention (+ output projection) -----
            with (
                tc.tile_pool(name="m", bufs=18) as m_pool,
                tc.tile_pool(name="p", bufs=8) as p_pool,
                tc.tile_pool(name="aux_sb", bufs=4) as aux_sb,
                tc.tile_pool(name="sc_ps", bufs=1, space="PSUM") as sc_ps,
                tc.tile_pool(name="av_ps", bufs=2, space="PSUM") as av_ps,
                tc.tile_pool(name="aux_ps", bufs=2, space="PSUM") as aux_ps,
            ):
                for qg in range(NSC):
                    msk = []
                    for kst in range(NST):
                        mt = m_pool.tile([128, 1024], BF16, tag="m", name="mt")
                        nc.sync.dma_start(out=mt, in_=mf[qg, kst])
                        msk.append(mt)
                    for pr in range(NJT):
                        qa = qt_sb[pr][0:64, qg * 512:(qg + 1) * 512]
                        qb = qt_sb[pr][64:128, qg * 512:(qg + 1) * 512]
                        pts = []
                        for kg in range(NST // 2):  # two ks-tiles per group
                            sab = sc_ps.tile([128, 2048], F32, tag="s", name="sab")
                            pt = p_pool.tile([128, 2048], BF16, tag="p", name="pt")
                            for u in range(2):
                                kst = 2 * kg + u
                                ka = kt_sb[pr][0:64, kst * 128:(kst + 1) * 128]
                                kb = kt_sb[pr][64:128, kst * 128:(kst + 1) * 128]
                                nc.tensor.matmul(sab[:, u * 1024:u * 1024 + 512],
                                                 ka, qa, tile_position=(0, 0))
                                nc.tensor.matmul(sab[:, u * 1024 + 512:(u + 1) * 1024],
                                                 kb, qb, tile_position=(64, 0))
                            nc.scalar.activation(out=pt, in_=sab, func=Exp, scale=0.125)
                            nc.vector.tensor_mul(pt[:, 0:1024], pt[:, 0:1024],
                                                 msk[2 * kg])
                            nc.vector.tensor_mul(pt[:, 1024:2048], pt[:, 1024:2048],
                                                 msk[2 * kg + 1])
                            pts.append(pt)
                        # A@V with denominator row (V ones-column)
                        otps = []
                        for hh in range(2):
                            h = 2 * pr + hh
                            ot = av_ps.tile([HD + 1, 512], F32, tag="av", name="ot")
                            for kst in range(NST):
                                nc.tensor.matmul(
                                    ot, v_sb[kst][:, h, :],
                                    pts[kst // 2][:, (kst % 2) * 1024 + hh * 512:
                                                  (kst % 2) * 1024 + (hh + 1) * 512],
                                    start=(kst == 0), stop=(kst == NST - 1))
                            otf = aux_sb.tile([HD + 1, 512], F32, tag="otf", name="otf")
                            nc.vector.tensor_copy(out=otf, in_=ot)
                            otps.append(otf)
                        # transpose -> normalize -> transpose back, 128 q at a time
                        for i in range(4):
                            onp = aux_sb.tile([128, 2 * HD], BF16, tag="on", name="onp")
                            for hh in range(2):
                                op = aux_ps.tile([128, HD + 1], F32, tag="aux", name="op")
                                nc.tensor.transpose(
                                    op, otps[hh][:, i * 128:(i + 1) * 128],
                                    id32_sb[0:HD + 1, 0:HD + 1])
                                rden = aux_sb.tile([128, 1], F32, tag="rd", name="rden")
                                nc.vector.reciprocal(out=rden, in_=op[:, HD:HD + 1])
                                nc.vector.tensor_scalar_mul(
                                    onp[:, hh * HD:(hh + 1) * HD], op[:, 0:HD], rden)
                            bk_ps = aux_ps.tile([128, 128], BF16, tag="aux", name="bkp")
                            nc.tensor.transpose(bk_ps, onp, id16_sb)
                            col = qg * 512 + i * 128
                            nc.vector.tensor_copy(
                                out=ot_sb[pr][:, col:col + 128], in_=bk_ps)

            # ---------------- phase 3: output projection ----------------
            with (
                tc.tile_pool(name="y_ps", bufs=4, space="PSUM") as y_ps,
                tc.tile_pool(name="y_sb", bufs=4) as y_sb,
            ):
                for qt in range(NST):
                    for nh in range(2):
                        ps = y_ps.tile([128, 512], F32, tag="y", name="yps")
                        for pr in range(NJT):
                            nc.tensor.matmul(
                                ps, ot_sb[pr][:, qt * 128:(qt + 1) * 128],
                                wo_sb[:, pr, nh * 512:(nh + 1) * 512],
                                start=(pr == 0), stop=(pr == NJT - 1))
                        ysb = y_sb.tile([128, 512], F32, tag="ysb", name="ysb")
                        nc.vector.tensor_copy(out=ysb, in_=ps)
                        nc.sync.dma_start(
                            out=yp[qt * 128:(qt + 1) * 128,
                                   nh * 512:(nh + 1) * 512],
                            in_=ysb)

    return nc


# --------------------------------------------------------------------------
def _prep_inputs(inputs):
    """Slice/transpose/cast the full inputs into 8 per-core input maps."""
    q_in = np.asarray(inputs["query_input"], np.float32)
    k_in = np.asarray(inputs["key_input"], np.float32)
    v_in = np.asarray(inputs["value_input"], np.float32)
    mask = np.asarray(inputs["mask"])  # [B, 1, S, S] bool
    Wq = np.asarray(inputs["Wq"], np.float32)
    Wk = np.asarray(inputs["Wk"], np.float32)
    Wv = np.asarray(inputs["Wv"], np.float32)
    Wo = np.asarray(inputs["Wo"], np.float32)
    bqf = np.asarray(inputs["bq"], np.float32)
    bkf = np.asarray(inputs["bk"], np.float32)
    bvf = np.asarray(inputs["bv"], np.float32)

    def tile_xt(x_b):
        # [S, D] -> x.T tiled [NSC, NK, 128, 512]
        xt = np.ascontiguousarray(x_b.T).astype(NP_BF16)        # [D, S]
        return np.ascontiguousarray(
            xt.reshape(NK, 128, NSC, 512).transpose(2, 0, 1, 3))

    xts = {n: [tile_xt(a[b]) for b in range(B)]
           for n, a in (("xq", q_in), ("xk", k_in), ("xv", v_in))}
    mfs = []
    for b in range(B):
        m = (~mask[b, 0]).T.astype(NP_BF16)                     # [ks, q]
        mt = m.reshape(NST, 128, NSC, 512).transpose(2, 0, 1, 3)
        mfs.append(np.ascontiguousarray(
            np.concatenate([mt, mt], axis=3)))              # duplicated for 2 heads

    in_maps = []
    for c in range(8):
        b, g = c // 2, c % 2
        js = slice(g * JC, (g + 1) * JC)
        in_maps.append({
            "xq": xts["xq"][b],
            "xk": xts["xk"][b],
            "xv": xts["xv"][b],
            "mf": mfs[b],
            "wq": np.ascontiguousarray(Wq[js, :].T).astype(NP_BF16).reshape(NK, 128, JC),
            "wk": np.ascontiguousarray(Wk[js, :].T).astype(NP_BF16).reshape(NK, 128, JC),
            "wv": np.ascontiguousarray(Wv[js, :].T).astype(NP_BF16).reshape(NK, 128, JC),
            "wo": np.ascontiguousarray(Wo[:, js].T).astype(NP_BF16).reshape(NJT, 128, D),
            "bq": np.ascontiguousarray(bqf[js].reshape(NJT, 128).T),
            "bk": np.ascontiguousarray(bkf[js].reshape(NJT, 128).T),
            "bv": np.ascontiguousarray(bvf[js].astype(NP_BF16).reshape(1, JC)),
        })
    return in_maps


def kernel(**inputs):
    _install_birfix()
    if "nc" not in _cache:
        _cache["nc"] = build_program()
    nc = _cache["nc"]
    in_maps = _prep_inputs(inputs)
    res = run_bass_kernel_spmd(nc, in_maps, core_ids=list(range(8)))
    bo = np.asarray(inputs["bo"], np.float32)
    out = np.empty((B, S, D), np.float32)
    for b in range(B):
        out[b] = res.results[2 * b]["yp"] + res.results[2 * b + 1]["yp"] + bo[None, :]
    return out


# revision 10
# speedup vs baseline: 1.1477x; 1.1281x over previous
"""Multi-headed attention (B=4, S=2048, D=1024, H=16) on 8 trn2 NeuronCores.

Sharding: core c handles batch b=c//2 and head-group g=c%2 (8 heads = 512 of
the 1024 projection output dims; data parallel over B, tensor parallel over
heads).  Each core computes its heads' Q/K/V projections, masked-softmax
attention, and a partial output projection (contraction over its 512
head-dims).  The host sums the two partials per batch and adds bo.

Layout strategy (all matmuls bf16 with fp32 PSUM accumulation):
  - activations kept feature-major (transposed): QT/KT [j, s]; V seq-major
    [s, j] with a ones-column appended per head so the A@V matmul emits the
    softmax denominator as a 65th output row for free.
  - scoresT [ks, q] per head; 2 heads run concurrently on the PE via
    tile_position row-tiling (K=64 each).
  - Exp on ScalarE straight out of PSUM (scale=1/sqrt(64) folded in; scores
    are O(1) so no max-subtraction is needed); mask applied multiplicatively
    on VectorE in bf16.
  - attention output OT [65, q] is PE-transposed to [q, 65], normalized by
    the reciprocal denominator (per-partition scalar), and PE-transposed
    back so the output projection can contract over head-dims.
"""

import json
import numpy as np
import ml_dtypes

import concourse.bass as bass
import concourse.mybir as mybir
import concourse.tile as tile
from concourse.bass_utils import run_bass_kernel_spmd
from concourse.masks import make_identity

BF16 = mybir.dt.bfloat16
F32 = mybir.dt.float32
NP_BF16 = ml_dtypes.bfloat16

B, S, D, H = 4, 2048, 1024, 16
HD = 64          # head dim
G = 2            # head groups (cores per batch)
HPC = H // G     # 8 heads per core
JC = HPC * HD    # 512 local projection dims per core
NK = D // 128    # 8 contraction chunks
NJT = JC // 128  # 4 j-tiles (head pairs)
NSC = S // 512   # 4 s/q groups of 512
NST = S // 128   # 16 s/ks tiles of 128

_cache = {}


# --------------------------------------------------------------------------
# walrus in this container encodes at most ONE sync-wait command per
# instruction; the Tile tail drain carries several.  Split them at the BIR
# JSON level into single-wait drains.
def _split_multi_waits(bir_json_bytes):
    d = json.loads(bir_json_bytes)

    def fix_block(bb):
        insts = bb.get("instructions")
        if not isinstance(insts, list):
            return
        out = []
        for ins in insts:
            si = ins.get("sync_info") or {}
            waits = si.get("on_wait") or []
            if len(waits) > 1:
                for i, w in enumerate(waits[:-1]):
                    out.append({
                        "debug": ins.get("debug"),
                        "engine": ins["engine"],
                        "ins": [],
                        "name": f"{ins['name']}-w{i}",
                        "opcode": "Drain",
                        "outs": [],
                        "sync_info": {"on_update": [], "on_wait": [w]},
                    })
                si["on_wait"] = [waits[-1]]
            out.append(ins)
        bb["instructions"] = out

    def walk(o):
        if isinstance(o, dict):
            if "instructions" in o:
                fix_block(o)
            for v in o.values():
                walk(v)
        elif isinstance(o, list):
            for v in o:
                walk(v)

    walk(d)
    return json.dumps(d).encode()


def _install_birfix():
    import concourse.bass_utils as bu
    if getattr(bu, "_birfix_installed", False):
        return
    orig = bu.compile_bir_kernel

    def patched(bir_json, tmpdir, neff_name="file.neff"):
        if isinstance(bir_json, str):
            bir_json = bir_json.encode()
        return orig(_split_multi_waits(bir_json), tmpdir, neff_name)

    bu.compile_bir_kernel = patched
    try:
        import concourse.bass2jax as b2j
        b2j.compile_bir_kernel = patched
    except Exception:
        pass
    bu._birfix_installed = True


# --------------------------------------------------------------------------
def build_program():
    nc = bass.Bass()
    Exp = mybir.ActivationFunctionType.Exp
    Ident = mybir.ActivationFunctionType.Identity

    xq = nc.declare_dram_parameter("xq", [NSC, NK, 128, 512], BF16, isOutput=False)
    xk = nc.declare_dram_parameter("xk", [NSC, NK, 128, 512], BF16, isOutput=False)
    xv = nc.declare_dram_parameter("xv", [NSC, NK, 128, 512], BF16, isOutput=False)
    mf = nc.declare_dram_parameter("mf", [NSC, NST, 128, 1024], BF16, isOutput=False)
    wq = nc.declare_dram_parameter("wq", [NK, 128, JC], BF16, isOutput=False)
    wk = nc.declare_dram_parameter("wk", [NK, 128, JC], BF16, isOutput=False)
    wv = nc.declare_dram_parameter("wv", [NK, 128, JC], BF16, isOutput=False)
    wo = nc.declare_dram_parameter("wo", [NJT, 128, D], BF16, isOutput=False)
    bq = nc.declare_dram_parameter("bq", [128, NJT], F32, isOutput=False)
    bk = nc.declare_dram_parameter("bk", [128, NJT], F32, isOutput=False)
    bv = nc.declare_dram_parameter("bv", [1, JC], BF16, isOutput=False)
    yp = nc.declare_dram_parameter("yp", [S, D], F32, isOutput=True)

    with tile.TileContext(nc) as tc:
        with tc.tile_pool(name="singles", bufs=1) as singles:
            qt_sb = [singles.tile([128, S], BF16, tag=f"qt{j}", name=f"qt{j}") for j in range(NJT)]
            kt_sb = [singles.tile([128, S], BF16, tag=f"kt{j}", name=f"kt{j}") for j in range(NJT)]
            v_sb = [singles.tile([128, HPC, HD + 1], BF16, tag=f"v{t}", name=f"v{t}") for t in range(NST)]
            ot_sb = [singles.tile([128, S], BF16, tag=f"ot{j}", name=f"ot{j}") for j in range(NJT)]
            wq_sb = singles.tile([128, NK, JC], BF16, tag="wq")
            wk_sb = singles.tile([128, NK, JC], BF16, tag="wk")
            wv_sb = singles.tile([128, NK, JC], BF16, tag="wv")
            wo_sb = singles.tile([128, NJT, D], BF16, tag="wo")
            bq_sb = singles.tile([128, NJT], F32, tag="bq")
            bk_sb = singles.tile([128, NJT], F32, tag="bk")
            bv_sb = singles.tile([1, JC], BF16, tag="bv")
            ones_sb = singles.tile([1, 128], BF16, tag="ones")
            id32_sb = singles.tile([128, 128], F32, tag="id32")
            id16_sb = singles.tile([128, 128], BF16, tag="id16")

            nc.sync.dma_start(out=wv_sb, in_=wv.rearrange("k p n -> p k n"))
            nc.sync.dma_start(out=wk_sb, in_=wk.rearrange("k p n -> p k n"))
            nc.sync.dma_start(out=wq_sb, in_=wq.rearrange("k p n -> p k n"))
            nc.sync.dma_start(out=wo_sb, in_=wo.rearrange("c p n -> p c n"))
            nc.sync.dma_start(out=bq_sb, in_=bq[:, :])
            nc.sync.dma_start(out=bk_sb, in_=bk[:, :])
            nc.sync.dma_start(out=bv_sb, in_=bv[:, :])
            nc.vector.memset(ones_sb, 1.0)
            make_identity(nc, id32_sb)
            make_identity(nc, id16_sb)
            for t in range(NST):
                nc.vector.memset(v_sb[t][:, :, HD:HD + 1], 1.0)

            # ---------------- phase 1: projections ----------------
            # V first (attention needs all of it), then K/Q by j-tile so the
            # first head-pair's attention can start while later pairs project.
            with (
                tc.tile_pool(name="xv_p", bufs=4) as xv_pool,
                tc.tile_pool(name="xq_p", bufs=4 * NK) as xq_pool,
                tc.tile_pool(name="xk_p", bufs=4 * NK) as xk_pool,
                tc.tile_pool(name="pj_ps", bufs=8, space="PSUM") as pj_ps,
            ):
                for sc in range(NSC):
                    ps = [pj_ps.tile([128, 512], F32, tag="pj", name="pj") for _ in range(4)]
                    for k in range(NK):
                        xt = xv_pool.tile([128, 512], BF16, tag="xv", name="xvt")
                        nc.sync.dma_start(out=xt, in_=xv[sc, k])
                        for sl in range(4):
                            nc.tensor.matmul(
                                ps[sl], xt[:, sl * 128:(sl + 1) * 128], wv_sb[:, k, :],
                                start=(k == 0), stop=False)
                    for sl in range(4):
                        nc.tensor.matmul(ps[sl], ones_sb, bv_sb, start=False, stop=True)
                        nc.vector.tensor_copy(
                            out=v_sb[sc * 4 + sl][:, :, 0:HD], in_=ps[sl])

                # load all xq/xk tiles once; loop j-tiles outermost
                xqt = {}
                xkt = {}
                for sc in range(NSC):
                    for k in range(NK):
                        t1 = xk_pool.tile([128, 512], BF16, tag="xk", name="xkt")
                        nc.sync.dma_start(out=t1, in_=xk[sc, k])
                        xkt[sc, k] = t1
                        t2 = xq_pool.tile([128, 512], BF16, tag="xq", name="xqt")
                        nc.sync.dma_start(out=t2, in_=xq[sc, k])
                        xqt[sc, k] = t2
                for jt in range(NJT):
                    for w_t, b_t, dst, xtiles in (
                        (wk_sb, bk_sb, kt_sb, xkt),
                        (wq_sb, bq_sb, qt_sb, xqt),
                    ):
                        for sc in range(NSC):
                            ps = pj_ps.tile([128, 512], F32, tag="pj", name="pj")
                            for k in range(NK):
                                nc.tensor.matmul(
                                    ps, w_t[:, k, jt * 128:(jt + 1) * 128],
                                    xtiles[sc, k],
                                    start=(k == 0), stop=(k == NK - 1))
                            nc.vector.tensor_scalar_add(
                                dst[jt][:, sc * 512:(sc + 1) * 512], ps,
                                b_t[:, jt:jt + 1])

            # ---------------- phase 2: attention (+ output projection) -----
            with (
                tc.tile_pool(name="m", bufs=18) as m_pool,
                tc.tile_pool(name="p", bufs=18) as p_pool,
                tc.tile_pool(name="aux_sb", bufs=4) as aux_sb,
                tc.tile_pool(name="sc_ps", bufs=2, space="PSUM") as sc_ps,
                tc.tile_pool(name="av_ps", bufs=2, space="PSUM") as av_ps,
                tc.tile_pool(name="aux_ps", bufs=2, space="PSUM") as aux_ps,
            ):
                for qg in range(NSC):
                    msk = []
                    for kst in range(NST):
                        mt = m_pool.tile([128, 1024], BF16, tag="m", name="mt")
                        nc.sync.dma_start(out=mt, in_=mf[qg, kst])
                        msk.append(mt)
                    for pr in range(NJT):
                        qa = qt_sb[pr][0:64, qg * 512:(qg + 1) * 512]
                        qb = qt_sb[pr][64:128, qg * 512:(qg + 1) * 512]
                        pts = []
                        for kst in range(NST):
                            sab = sc_ps.tile([128, 1024], F32, tag="s", name="sab")
                            pt = p_pool.tile([128, 1024], BF16, tag="p", name="pt")
                            ka = kt_sb[pr][0:64, kst * 128:(kst + 1) * 128]
                            kb = kt_sb[pr][64:128, kst * 128:(kst + 1) * 128]
                            nc.tensor.matmul(sab[:, 0:512], ka, qa,
                                             tile_position=(0, 0))
                            nc.tensor.matmul(sab[:, 512:1024], kb, qb,
                                             tile_position=(64, 0))
                            nc.scalar.activation(out=pt, in_=sab, func=Exp, scale=0.125)
                            nc.vector.tensor_mul(pt, pt, msk[kst])
                            pts.append(pt)
                        # A@V with denominator row (V ones-column)
                        otps = []
                        for hh in range(2):
                            h = 2 * pr + hh
                            ot = av_ps.tile([HD + 1, 512], F32, tag="av", name="ot")
                            for kst in range(NST):
                                nc.tensor.matmul(
                                    ot, v_sb[kst][:, h, :],
                                    pts[kst][:, hh * 512:(hh + 1) * 512],
                                    start=(kst == 0), stop=(kst == NST - 1))
                            otf = aux_sb.tile([HD + 1, 512], F32, tag="otf", name="otf")
                            nc.vector.tensor_copy(out=otf, in_=ot)
                            otps.append(otf)
                        # transpose -> normalize -> transpose back, 128 q at a time
                        for i in range(4):
                            onp = aux_sb.tile([128, 2 * HD], BF16, tag="on", name="onp")
                            for hh in range(2):
                                op = aux_ps.tile([128, HD + 1], F32, tag="aux", name="op")
                                nc.tensor.transpose(
                                    op, otps[hh][:, i * 128:(i + 1) * 128],
                                    id32_sb[0:HD + 1, 0:HD + 1])
                                rden = aux_sb.tile([128, 1], F32, tag="rd", name="rden")
                                nc.vector.reciprocal(out=rden, in_=op[:, HD:HD + 1])
                                nc.vector.tensor_scalar_mul(
                                    onp[:, hh * HD:(hh + 1) * HD], op[:, 0:HD], rden)
                            bk_ps = aux_ps.tile([128, 128], BF16, tag="aux", name="bkp")
                            nc.tensor.transpose(bk_ps, onp, id16_sb)
                            col = qg * 512 + i * 128
                            nc.vector.tensor_copy(
                                out=ot_sb[pr][:, col:col + 128], in_=bk_ps)

            # ---------------- phase 3: output projection ----------------
            with (
                tc.tile_pool(name="y_ps", bufs=4, space="PSUM") as y_ps,
                tc.tile_pool(name="y_sb", bufs=4) as y_sb,
            ):
                for qt in range(NST):
                    for nh in range(2):
                        ps = y_ps.tile([128, 512], F32, tag="y", name="yps")
                        for pr in range(NJT):
                            nc.tensor.matmul(
                                ps, ot_sb[pr][:, qt * 128:(qt + 1) * 128],
                                wo_sb[:, pr, nh * 512:(nh + 1) * 512],
                                start=(pr == 0), stop=(pr == NJT - 1))
                        ysb = y_sb.tile([128, 512], F32, tag="ysb", name="ysb")
                        nc.vector.tensor_copy(out=ysb, in_=ps)
                        nc.sync.dma_start(
                            out=yp[qt * 128:(qt + 1) * 128,
                                   nh * 512:(nh + 1) * 512],
                            in_=ysb)

    return nc


# --------------------------------------------------------------------------
def _prep_inputs(inputs):
    """Slice/transpose/cast the full inputs into 8 per-core input maps."""
    q_in = np.asarray(inputs["query_input"], np.float32)
    k_in = np.asarray(inputs["key_input"], np.float32)
    v_in = np.asarray(inputs["value_input"], np.float32)
    mask = np.asarray(inputs["mask"])  # [B, 1, S, S] bool
    Wq = np.asarray(inputs["Wq"], np.float32)
    Wk = np.asarray(inputs["Wk"], np.float32)
    Wv = np.asarray(inputs["Wv"], np.float32)
    Wo = np.asarray(inputs["Wo"], np.float32)
    bqf = np.asarray(inputs["bq"], np.float32)
    bkf = np.asarray(inputs["bk"], np.float32)
    bvf = np.asarray(inputs["bv"], np.float32)

    def tile_xt(x_b):
        # [S, D] -> x.T tiled [NSC, NK, 128, 512]
        xt = np.ascontiguousarray(x_b.T).astype(NP_BF16)        # [D, S]
        return np.ascontiguousarray(
            xt.reshape(NK, 128, NSC, 512).transpose(2, 0, 1, 3))

    xts = {n: [tile_xt(a[b]) for b in range(B)]
           for n, a in (("xq", q_in), ("xk", k_in), ("xv", v_in))}
    mfs = []
    for b in range(B):
        m = (~mask[b, 0]).T.astype(NP_BF16)                     # [ks, q]
        mt = m.reshape(NST, 128, NSC, 512).transpose(2, 0, 1, 3)
        mfs.append(np.ascontiguousarray(
            np.concatenate([mt, mt], axis=3)))              # duplicated for 2 heads

    in_maps = []
    for c in range(8):
        b, g = c // 2, c % 2
        js = slice(g * JC, (g + 1) * JC)
        in_maps.append({
            "xq": xts["xq"][b],
            "xk": xts["xk"][b],
            "xv": xts["xv"][b],
            "mf": mfs[b],
            "wq": np.ascontiguousarray(Wq[js, :].T).astype(NP_BF16).reshape(NK, 128, JC),
            "wk": np.ascontiguousarray(Wk[js, :].T).astype(NP_BF16).reshape(NK, 128, JC),
            "wv": np.ascontiguousarray(Wv[js, :].T).astype(NP_BF16).reshape(NK, 128, JC),
            "wo": np.ascontiguousarray(Wo[:, js].T).astype(NP_BF16).reshape(NJT, 128, D),
            "bq": np.ascontiguousarray(bqf[js].reshape(NJT, 128).T),
            "bk": np.ascontiguousarray(bkf[js].reshape(NJT, 128).T),
            "bv": np.ascontiguousarray(bvf[js].astype(NP_BF16).reshape(1, JC)),
        })
    return in_maps


def kernel(**inputs):
    _install_birfix()
    if "nc" not in _cache:
        _cache["nc"] = build_program()
    nc = _cache["nc"]
    in_maps = _prep_inputs(inputs)
    res = run_bass_kernel_spmd(nc, in_maps, core_ids=list(range(8)))
    bo = np.asarray(inputs["bo"], np.float32)
    out = np.empty((B, S, D), np.float32)
    for b in range(B):
        out[b] = res.results[2 * b]["yp"] + res.results[2 * b + 1]["yp"] + bo[None, :]
    return out
